# revision 1
# baseline (speedup 1.0000x reference)
"""Ternary (BitwiseLinear) matmul kernel for Trainium2, 8-core data-parallel.

y = ternary(x) @ ternary(w).T  with threshold 0.05, int-exact accumulation.

Sharding: x is split along the token dim across 8 cores (4096 tokens each);
the weight is replicated. Each core computes its y shard independently
(no collectives) and shards are concatenated on the host.

Per-core pipeline (v2):
  1. quantize to ternary: u = (v>=T) on DVE, vneg = (v<=-T) on GPSIMD,
     q = u - vneg on DVE -> bf16 {-1,0,1}. Exact (no element == +-T).
  2. PE-transpose q 128x128 blocks into one bf16 PSUM bank; evict with a
     single DVE/ACT copy per tile, casting to fp8e4 -> k-major layout.
  3. fp8 DoubleRow matmuls (K=256 per instruction) accumulate y tile
     [t:128, o:2x512] in a 2-bank PSUM tile, evict DVE/ACT, DMA out.
  4. DMAs batched >=1MB; all on sync (HWDGE) to keep GPSIMD free.
"""

import contextlib
import threading

import numpy as np

N_CORES = 8
TOKENS = 32768
TOK_PER_CORE = TOKENS // N_CORES
K = 1024
O = 1024
P = 128
THR = 0.05

_cache = {}
_lock = threading.Lock()


def _split_multi_waits(nc):
    """walrus in this env can't encode >1 sync wait on one instruction: hoist
    extra waits into single-wait NOPs on the same engine, just before the
    instruction (identical per-engine wait semantics)."""
    import concourse.mybir as mybir

    uid = 0
    for f in nc.m.functions:
        for b in f.blocks:
            out = []
            changed = False
            for inst in b.instructions:
                si = inst.sync_info
                if si is not None and si.on_wait and len(si.on_wait) > 1:
                    waits = list(si.on_wait)
                    for w in waits[:-1]:
                        uid += 1
                        out.append(mybir.InstNoOp(
                            name=f"I-waitsplit-{uid}",
                            engine=inst.engine,
                            sync_info=mybir.SyncInfo(on_wait=[w], on_update=[]),
                        ))
                    inst.sync_info = mybir.SyncInfo(
                        on_wait=[waits[-1]], on_update=list(si.on_update))
                    changed = True
                out.append(inst)
            if changed:
                b.instructions = out


def build_nc(tokens=TOK_PER_CORE, loop_n=1, skip_transpose=False, skip_mm=False, skip_quant=False):
    import concourse.bass as bass
    import concourse.mybir as mybir
    from concourse.masks import make_identity
    from concourse.tile import TileContext

    F32 = mybir.dt.float32
    BF16 = mybir.dt.bfloat16
    FP8 = mybir.dt.float8e4
    A = mybir.AluOpType

    KB = K // P          # 8 k-blocks of 128
    n_ttiles = tokens // P

    nc = bass.Bass()
    x = nc.dram_tensor("x", [tokens, K], F32, kind="ExternalInput")
    w = nc.dram_tensor("weight", [O, K], F32, kind="ExternalInput")
    y = nc.dram_tensor("out", [tokens, O], F32, kind="ExternalOutput")

    # [t, k] viewed as [t-pair, p, k] for 2-tile-batched loads
    x2 = x.rearrange("(a p) k -> a p k", p=P)   # a = row-block index
    w2 = w.rearrange("(a p) k -> a p k", p=P)

    with TileContext(nc) as tc:
        with (
            tc.tile_pool(name="const", bufs=1) as const_pool,
            tc.tile_pool(name="wqt", bufs=1) as wqt_pool,
            tc.tile_pool(name="xin", bufs=3) as xin_pool,
            tc.tile_pool(name="quant", bufs=3) as q_pool,
            tc.tile_pool(name="xqt", bufs=3) as xqt_pool,
            tc.tile_pool(name="yout", bufs=3) as y_pool,
            tc.tile_pool(name="psum_t", bufs=2, space="PSUM") as psumt_pool,
            tc.tile_pool(name="psum_y", bufs=3, space="PSUM") as psumy_pool,
        ):
            identity = const_pool.tile([P, P], BF16)
            make_identity(nc, identity)

            def quantize(src):
                """f32 [128, K] view -> ternary bf16 [128, K]."""
                if skip_quant:
                    q = q_pool.tile([P, K], BF16, tag="q_q")
                    return q
                u = q_pool.tile([P, K], BF16, tag="q_u")
                nc.vector.tensor_scalar(
                    out=u[:], in0=src, scalar1=THR, scalar2=None, op0=A.is_ge)
                v = q_pool.tile([P, K], BF16, tag="q_v")
                nc.vector.tensor_scalar(
                    out=v[:], in0=src, scalar1=-THR, scalar2=None, op0=A.is_le)
                q = q_pool.tile([P, K], BF16, tag="q_q")
                nc.vector.tensor_tensor(out=q[:], in0=u[:], in1=v[:],
                                        op=A.subtract)
                return q

            def transpose_to(q, dst, evict_engine):
                """q bf16 [128, K] natural -> dst fp8 [128, KB, 128] k-major."""
                ps = psumt_pool.tile([P, KB, P], BF16, tag="psT")
                if not skip_transpose:
                    for kb in range(KB):
                        nc.tensor.transpose(
                            ps[:, kb, :], q[:, kb * P:(kb + 1) * P], identity)
                if evict_engine == "vector":
                    nc.vector.tensor_copy(dst[:], ps[:])
                else:
                    nc.scalar.copy(dst[:], ps[:])

            # --- weight phase: wqT fp8 [k_part, k_blk, o] ---
            wqT = wqt_pool.tile([P, KB, O], FP8)
            for pair in range(O // (2 * P)):       # 4 batched loads
                wt = xin_pool.tile([P, 2, K], F32, tag="w_in")
                nc.sync.dma_start(
                    wt[:], w2[2 * pair:2 * pair + 2].rearrange("a p k -> p a k"))
                for j in range(2):
                    ob = 2 * pair + j
                    qw = quantize(wt[:, j, :])
                    transpose_to(qw, wqT[:, :, ob * P:(ob + 1) * P], "scalar")

            def main_body():
                for tp in range(n_ttiles // 2):    # pairs of token tiles
                    xt = xin_pool.tile([P, 2, K], F32, tag="x_in")
                    nc.sync.dma_start(
                        xt[:],
                        x2[2 * tp:2 * tp + 2].rearrange("a p k -> p a k"))
                    for j in range(2):
                        tb = 2 * tp + j
                        qx = quantize(xt[:, j, :])
                        xqT = xqt_pool.tile([P, KB, P], FP8, tag="xqT")
                        transpose_to(qx, xqT, "scalar")

                        yp = psumy_pool.tile([P, 2, 512], F32, tag="yp")
                        for oh in ([] if skip_mm else range(2)):
                            for s in range(KB // 2):   # 4 DoubleRow steps
                                nc.tensor.matmul(
                                    yp[:, oh, :],
                                    xqT[:, 2 * s:2 * s + 2, :],
                                    wqT[:, 2 * s:2 * s + 2,
                                        oh * 512:(oh + 1) * 512],
                                    start=(s == 0),
                                    stop=(s == KB // 2 - 1),
                                    perf_mode=mybir.MatmulPerfMode.DoubleRow,
                                )
                        ysb = y_pool.tile([P, K], F32, tag="ysb")
                        nc.scalar.copy(ysb[:], yp[:])
                        nc.sync.dma_start(y[tb * P:(tb + 1) * P, :], ysb[:])

            # loop_n > 1 wraps the token loop in a hardware loop purely for
            # benchmarking (amortizes per-call host/PJRT overhead).
            if loop_n > 1:
                with tc.For_i(0, loop_n, 1):
                    main_body()
            else:
                main_body()

    _split_multi_waits(nc)
    return nc


def _get_nc(tokens=TOK_PER_CORE):
    with _lock:
        if tokens not in _cache:
            _cache[tokens] = build_nc(tokens)
        return _cache[tokens]


def kernel(x: np.ndarray, weight: np.ndarray):
    from concourse.bass_utils import run_bass_kernel_spmd

    x = np.ascontiguousarray(x, dtype=np.float32)
    weight = np.ascontiguousarray(weight, dtype=np.float32)
    assert x.shape == (TOKENS, K) and weight.shape == (O, K)

    nc = _get_nc()
    in_maps = [
        {"x": x[i * TOK_PER_CORE:(i + 1) * TOK_PER_CORE], "weight": weight}
        for i in range(N_CORES)
    ]
    res = run_bass_kernel_spmd(nc, in_maps, core_ids=list(range(N_CORES)))
    return np.concatenate([r["out"] for r in res.results], axis=0)

